# revision 16
# baseline (speedup 1.0000x reference)
"""Conv2d 3x3 (stride 1, pad 1) NCHW kernel for 8 Trainium2 NeuronCores.

Problem: x (32,128,56,56) f32, weight (256,128,3,3), bias (256,)
         -> out (32,256,56,56), same-padding conv + bias.

Strategy:
  - Data parallel: 4 images per core across 8 cores (batch shard).
  - fp8 DoubleRow implicit GEMM.  Each operand is split hi/lo into two
    fp8e4m3 planes (x ~ x_hi + x_lo, w ~ w_hi + w_lo); keeping the
    x_hi*w_hi, x_hi*w_lo and x_lo*w_hi products reproduces the f32 conv
    to ~1.3e-3 rel err.  A DoubleRow matmul contracts TWO independent
    128-deep products per output column at 0.5 cycles/row, so the 27
    tap-products per output tile fit in 14 matmuls -- 0.78x the PE time
    of the 9-tap f32r formulation, at 4x per-product throughput.
  - Layout trick: the padded image (58x58) is stored row-contiguous, so
    the moving operand for every tap is a single-stride 464-element
    window (8 output rows x 58).  Columns 56/57 of each PSUM row are
    junk (row wrap-around) and are simply never evicted/stored.  The
    hi/lo planes live at a fixed 3368-element offset inside one SBUF
    tile, which makes every hi/lo and tap-pair combination expressible
    as a 3D access pattern [128 x 2 x 464] with constant strides.
  - Bias is fused into the PSUM->SBUF eviction (ACT/DVE alternating).
    The final group's eviction/store is split across ACT+DVE and both
    HWDGE rings to shorten the kernel tail.
"""

import numpy as np

N_CORES = 8
N, C, H, W = 32, 128, 56, 56
O = 256
PAD = 1
HP = H + 2 * PAD  # 58
ROWB = W + 2 * PAD  # 58 elements per stored row
PLANE = 3368  # 58*58 = 3364 rounded up (pad matmul reads 2 past the end)
NPC = N // N_CORES  # images per core = 4
RPC = 8  # output rows per chunk
FREE = RPC * ROWB  # 464 moving columns per matmul
N_CHUNKS = H // RPC  # 7
OC_TILES = O // 128  # 2
NTAPS = 9
# tap t=(kh,kw) shifts the flat window by s(t) = kh*58 + kw
TAP_S = [(t // 3) * ROWB + (t % 3) for t in range(NTAPS)]
# weight tile layout per partition c: [oc_tile, plane(hi/lo), tap, m]
WOC = 2 * 10 * 128  # 2560 elements per oc tile
WPL = 10 * 128  # 1280 elements per plane
WTAP = 128

_CACHE = {}
LAST_RESULTS = None


def _build():
    import concourse.bass as bass
    import concourse.bacc as bacc
    import concourse.mybir as mybir
    import concourse.tile as tile
    from concourse.ap import AP

    f32 = mybir.dt.float32
    f8 = mybir.dt.float8e4
    DR = mybir.MatmulPerfMode.DoubleRow

    nc = bacc.Bacc(
        "TRN2", target_bir_lowering=False, debug=False, num_devices=N_CORES
    )
    xp_d = nc.dram_tensor("xp", (NPC, C, 2, PLANE), f8, kind="ExternalInput")
    w_d = nc.dram_tensor("w2", (C, OC_TILES, 2 * WPL), f8, kind="ExternalInput")
    b_d = nc.dram_tensor("b2", (128, OC_TILES), f32, kind="ExternalInput")
    out_d = nc.dram_tensor("out", (NPC, O, H, W), f32, kind="ExternalOutput")

    with tile.TileContext(nc) as tc:
        with (
            tc.tile_pool(name="w", bufs=1) as wpool,
            tc.tile_pool(name="x", bufs=2) as xpool,
            tc.tile_pool(name="ps", bufs=6, space=bass.MemorySpace.PSUM) as pspool,
            tc.tile_pool(name="psw", bufs=1, space=bass.MemorySpace.PSUM) as pspool_w,
            tc.tile_pool(name="o", bufs=6) as opool,
        ):
            w_t = wpool.tile([C, OC_TILES, 2 * WPL], f8)
            b_t = wpool.tile([128, OC_TILES], f32)
            # PE p-state warmup: the cost model runs matmuls at half rate
            # until the PE has been continuously busy for 3us.  A memset
            # tile + dummy DoubleRow matmuls start that clock at ~0.7us so
            # the ramp overlaps the startup DMAs and the real stream runs
            # warm from its first instruction.  36 dummies of free-dim 232
            # (~97ns each at the mid p-state) bridge to the first DMA sems.
            warm_t = wpool.tile([128, FREE], f8)
            warm_ps = pspool_w.tile([128, FREE], f32)
            nc.gpsimd.memset(warm_t[:], 0)
            warm_lhs = AP(
                warm_t.tensor, warm_t.offset, [[FREE, 128], [128, 2], [1, 128]]
            )
            warm_rhs = AP(
                warm_t.tensor, warm_t.offset, [[FREE, 128], [232, 2], [1, 232]]
            )
            warm_out = AP(warm_ps.tensor, warm_ps.offset, [[FREE, 128], [1, 232]])
            for _ in range(37):
                nc.tensor.matmul(
                    warm_out,
                    warm_lhs,
                    warm_rhs,
                    start=True,
                    stop=True,
                    perf_mode=DR,
                )
            # startup-critical DMAs in deadline order across three HWDGE
            # rings: oc0 weights on ACT, oc1 weights + bias on DVE, image 0
            # on SP (hi head / lo head / hi rest / lo rest).
            nc.scalar.dma_start(w_t[:, 0], w_d[:, 0])
            nc.gpsimd.dma_start(w_t[:, 1], w_d[:, 1])
            nc.gpsimd.dma_start(b_t[:], b_d[:])

            def wap(oc, off, d1):
                return AP(
                    w_t.tensor,
                    w_t.offset + oc * WOC + off,
                    [[OC_TILES * 2 * WPL, 128], [d1, 2], [1, 128]],
                )

            def group(x_t, ps_ap, base, oc, free=7 * ROWB + W):
                xoff = x_t.offset + base
                xst = 2 * PLANE

                def xap(off, d1):
                    return AP(x_t.tensor, xoff + off, [[xst, 128], [d1, 2], [1, free]])

                k = 0

                def mm(lhsT, rhs):
                    nonlocal k
                    nc.tensor.matmul(
                        ps_ap,
                        lhsT,
                        rhs,
                        start=(k == 0),
                        stop=(k == 13),
                        perf_mode=DR,
                    )
                    k += 1

                # 4 main-pair matmuls: (w_hi@t, w_hi@t+1) x (x_hi@t, x_hi@t+1)
                for t in (0, 2, 4, 6):
                    mm(
                        wap(oc, t * WTAP, WTAP),
                        xap(PLANE + TAP_S[t], TAP_S[t + 1] - TAP_S[t]),
                    )
                # main tap 8 paired with the all-zero tap 9 of the lo plane
                mm(wap(oc, 8 * WTAP, WPL + WTAP), xap(PLANE + TAP_S[8], 2))
                # 9 cross matmuls: (w_hi@t, w_lo@t) x (x_lo@t, x_hi@t)
                for t in range(NTAPS):
                    mm(wap(oc, t * WTAP, WPL), xap(TAP_S[t], PLANE))

            for idx in range(NPC):
                x_t = xpool.tile([C, 2, PLANE], f8)
                if idx == 0:
                    # image 0 on the SP ring: hi head gates the main pairs
                    # of chunk 0, lo head the cross matmuls; rests follow.
                    nc.sync.dma_start(x_t[:, 1, 0:640], xp_d[0, :, 1, 0:640])
                    nc.sync.dma_start(x_t[:, 0, 0:640], xp_d[0, :, 0, 0:640])
                    nc.sync.dma_start(x_t[:, 1, 640:2048], xp_d[0, :, 1, 640:2048])
                    nc.sync.dma_start(x_t[:, 0, 640:2048], xp_d[0, :, 0, 640:2048])
                    nc.sync.dma_start(x_t[:, 1, 2048:PLANE], xp_d[0, :, 1, 2048:PLANE])
                    nc.sync.dma_start(x_t[:, 0, 2048:PLANE], xp_d[0, :, 0, 2048:PLANE])
                else:
                    # later images ride the SP ring behind image 0 and the
                    # early stores; the manual wait keeps their 2.4us
                    # transfers from hoisting ahead of startup-critical DMAs.
                    with tc.tile_wait_until(0.005 * idx):
                        nc.sync.dma_start(x_t[:], xp_d[idx])
                for ch in range(N_CHUNKS):
                    base = ch * RPC * ROWB
                    for oc in range(OC_TILES):
                        bias_ap = b_t[:, oc : oc + 1]
                        out_ap = out_d[
                            idx, oc * 128 : (oc + 1) * 128, ch * RPC : (ch + 1) * RPC, :
                        ]
                        is_last = (
                            idx == NPC - 1 and ch == N_CHUNKS - 1 and oc == OC_TILES - 1
                        )
                        if is_last:
                            # final group runs as two half-height groups so
                            # the first half's eviction/store overlaps the
                            # second half's matmuls; the tail is quartered
                            # across ACT+DVE and both HWDGE rings with the
                            # last piece on the faster SP ring.
                            h = RPC // 2
                            psA = pspool.tile([128, RPC, ROWB], f32, tag="ps")
                            psA462 = AP(
                                psA.tensor, psA.offset,
                                [list(psA[:].ap[0]), [1, 3 * ROWB + W]],
                            )
                            group(x_t, psA462, base, oc, free=3 * ROWB + W)
                            o_tA = opool.tile([128, h, W], f32, tag="oh")
                            nc.scalar.add(o_tA[:], psA[:, 0:h, 0:W], bias_ap)
                            nc.sync.dma_start(out_ap[:, 0:h, :], o_tA[:])
                            psB = pspool.tile([128, RPC, ROWB], f32, tag="ps")
                            psB462 = AP(
                                psB.tensor, psB.offset,
                                [list(psB[:].ap[0]), [1, 3 * ROWB + W]],
                            )
                            group(x_t, psB462, base + h * ROWB, oc, free=3 * ROWB + W)
                            o_tB = opool.tile([128, h, W], f32, tag="oh")
                            nc.scalar.add(o_tB[:], psB[:, 0:h, 0:W], bias_ap)
                            nc.gpsimd.dma_start(out_ap[:, h:RPC, :], o_tB[:])
                            continue
                        ps = pspool.tile([128, RPC, ROWB], f32)
                        ps462 = AP(
                            ps.tensor, ps.offset, [list(ps[:].ap[0]), [1, 7 * ROWB + W]]
                        )
                        group(x_t, ps462, base, oc)
                        o_t = opool.tile([128, RPC, W], f32)
                        interior = ps[:, :, 0:W]
                        if (ch * OC_TILES + oc) % 2 == 0:
                            nc.scalar.add(o_t[:], interior, bias_ap)
                            nc.sync.dma_start(out_ap, o_t[:])
                        else:
                            nc.vector.tensor_scalar_add(o_t[:], interior, bias_ap)
                            nc.sync.dma_start(out_ap, o_t[:])
    nc.compile()
    return nc


def kernel(x, weight, bias):
    global LAST_RESULTS
    import ml_dtypes
    from concourse.bass_utils import run_bass_kernel_spmd

    f8 = ml_dtypes.float8_e4m3
    x = np.asarray(x, dtype=np.float32)
    weight = np.asarray(weight, dtype=np.float32)
    bias = np.asarray(bias, dtype=np.float32)

    # padded row-contiguous image planes: [N, C, 2(lo,hi), PLANE] fp8
    xpad = np.zeros((N, C, HP, ROWB), np.float32)
    xpad[:, :, PAD : PAD + H, PAD : PAD + W] = x
    xpad = xpad.reshape(N, C, HP * ROWB)
    x_hi = xpad.astype(f8)
    x_lo = (xpad - x_hi.astype(np.float32)).astype(f8)
    xp = np.zeros((N, C, 2, PLANE), f8)
    xp[:, :, 0, : HP * ROWB] = x_lo
    xp[:, :, 1, : HP * ROWB] = x_hi

    # weights: [C, oc_tile, plane(hi,lo), tap(10), m(128)] fp8, tap 9 = 0
    # wt[o, c, t] = weight[o, c, kh, kw]
    wt = weight.reshape(O, C, 9)
    w_hi = wt.astype(f8)
    w_lo = (wt - w_hi.astype(np.float32)).astype(f8)
    w2 = np.zeros((C, OC_TILES, 2, 10, 128), f8)
    # [o=oc*128+m, c, t] -> [c, oc, plane, t, m]
    w2[:, :, 0, :9, :] = w_hi.reshape(OC_TILES, 128, C, 9).transpose(2, 0, 3, 1)
    w2[:, :, 1, :9, :] = w_lo.reshape(OC_TILES, 128, C, 9).transpose(2, 0, 3, 1)
    w2 = w2.reshape(C, OC_TILES, 2 * WPL)

    b2 = np.ascontiguousarray(bias.reshape(OC_TILES, 128).T)

    if "nc" not in _CACHE:
        _CACHE["nc"] = _build()
    nc = _CACHE["nc"]

    in_maps = [
        {"xp": xp[i * NPC : (i + 1) * NPC], "w2": w2, "b2": b2}
        for i in range(N_CORES)
    ]
    res = run_bass_kernel_spmd(nc, in_maps, core_ids=list(range(N_CORES)))
    LAST_RESULTS = res
    return np.concatenate([r["out"] for r in res.results], axis=0)
